# revision 1
# baseline (speedup 1.0000x reference)
"""Trainium2 Bass kernel for the cross-batch retrieval contrastive loss.

Reference semantics per batch b:
  sent_mean = mean(sent_feat * masks)                      (host)
  v1   = conv1([bef^T; broadcast sent_mean])               -> (196, 512)
  MHA over 196 positions, out_proj                         -> (196, 512)
  mod  = conv2(o); ql = mod @ q_w^T + q_b                  -> (196, 512)
  kl   = aft @ k_w^T + k_b                                 -> (196, 512)
  logits[a,b,l,m] = ql[a,l,:] . kl[b,m,:]
  t2v[a,b] = mean_l max_m ; v2t[a,b] = mean_m max_l
  loss = symmetric InfoNCE on S = 0.5*(t2v+v2t)*exp(logit_scale)   (host)

Key algebraic restructurings (all host-side weight folding):
  - conv1 folds into the qkv projections: q = (Wq@W1a).bef + Wq.txt(a),
    so the v1 intermediate never exists on device.  The per-batch text
    contribution is an ACT-copy bias for q/k; for v it is deferred to
    the attention output via po += (32*txv) (x) z, because
    (po + c(x)z) * (1/z) = po/z + c.
  - out_proj, conv2, q_w and k_w^T all fold into ONE weight:
      logits = ql . (Wkl.aft) = ((Wkl@Wql@Wc2@Wo).ot) . aft
    so kl is never materialized: `aft` (already fp8 in SBUF) is the
    logits moving operand directly, and the post-attention front-end is
    a single projection ot -> qlw.
  - t2v: exact row-max on DVE over 3-bank PSUM groups (G=3 batching
    amortizes the PSUM-access + seq overhead per reduce).
  - v2t: log-sum-exp over the partition axis: ACT exp (scale=beta) of
    the same PSUM tiles -> PE colsum with the amask indicator
    (attributes q-rows to their batch) -> one ACT ln per key-pair ->
    small DVE add-reduce.  beta is host-calibrated from a norm bound so
    max |beta*X| ~ 13 and the LSE error is ~1e-5 relative (the fp8
    quantization error of ~5e-4 dominates).
  - softmax normalizers: all 8 heads' colsums land in one (8,196) PSUM
    tile; ONE reciprocal_approx_fast per batch replaces 32 full-precision
    DVE reciprocals (1.4us each) on the critical path.

Sharding: data-parallel over the query-batch axis 'a' (4 batches/core x
8 cores); aft/logits key side replicated. The final 32x32 InfoNCE runs
on the host in float64 (tiny).
"""

import numpy as np
import ml_dtypes

B, LV, LT, D, H = 32, 196, 40, 512, 8
NCORES = 8
AL = B // NCORES          # query batches per core
KT = D // 128             # 128-row feature tiles per 512-dim tensor
LSPLIT = [(0, 128), (128, 68)]   # 196 = 128 + 68
NQ = AL * LV              # 784 query position-rows per core
NKEY = B * LV             # 6272 key position-rows
TQ = (NQ + 127) // 128    # 7 stationary tiles over query rows
NBP = B // 2              # 16 key-batch pairs
W2 = 2 * LV               # batch-pair moving width
W2P = 400                 # fe fp8 tile stride (16B-aligned for DoubleRow)
RK = 256                  # low-rank factorization of the fused logits weight
RT = RK // 128            # its 128-row tiles
S_OT = 32.0               # ot fp8 scale (from the 1/32 colsum)
BF16 = ml_dtypes.bfloat16
F8 = ml_dtypes.float8_e4m3fn

_CACHE = {}


def _build_program(scal, reps=1):
    from contextlib import ExitStack
    import concourse.bacc as bacc
    import concourse.tile as tile
    from concourse import mybir

    f32 = mybir.dt.float32
    bf = mybir.dt.bfloat16
    f8 = mybir.dt.float8e4

    nc = bacc.Bacc("TRN2", target_bir_lowering=False, debug=False,
                   num_devices=NCORES)

    d = {
        "befT": nc.dram_tensor("befT", [128, KT, NQ], f8,
                               kind="ExternalInput").ap(),
        "aftT": nc.dram_tensor("aftT", [128, RT, NKEY], f8,
                               kind="ExternalInput").ap(),
        # per-batch per-partition biases for the fused q/k projections
        "txq": nc.dram_tensor("txq", [128, KT * AL], f32,
                              kind="ExternalInput").ap(),
        "txk": nc.dram_tensor("txk", [128, KT * AL], f32,
                              kind="ExternalInput").ap(),
        # txv selector-stationary: txv[r, (a*8+h)*64+c] = (r==h)*32*txv[a,h*64+c]
        "txv": nc.dram_tensor("txv", [8, AL * D], bf,
                              kind="ExternalInput").ap(),
        "bqlw": nc.dram_tensor("bqlw", [128, RT], f32,
                               kind="ExternalInput").ap(),
        "amask": nc.dram_tensor("amask", [128, TQ * AL], bf,
                                kind="ExternalInput").ap(),
        "hsel": nc.dram_tensor("hsel", [8, KT * 128], bf,
                               kind="ExternalInput").ap(),
        # raw reduction outputs; tiny host epilogue finishes t2v/v2t
        "out2": nc.dram_tensor("out2", [AL, NBP * W2], f32,
                               kind="ExternalOutput").ap(),
        "outrm": nc.dram_tensor("outrm", [128, TQ * B], bf,
                                kind="ExternalOutput").ap(),
    }
    for n in ["wq18", "wk18", "wv18"]:
        d[n] = nc.dram_tensor(n, [128, KT, D], f8, kind="ExternalInput").ap()
    d["wqlw8"] = nc.dram_tensor("wqlw8", [128, KT, RK], f8,
                                kind="ExternalInput").ap()

    with tile.TileContext(nc) as tc, ExitStack() as ctx:
        const = ctx.enter_context(tc.tile_pool(name="const", bufs=1))
        big = ctx.enter_context(tc.tile_pool(name="big", bufs=1))
        fe = ctx.enter_context(tc.tile_pool(name="fe", bufs=2))
        # PSUM budget (8 banks): a1 3x1 + pzs 1x1 + g2 2x2
        ps = ctx.enter_context(tc.tile_pool(name="ps", bufs=2, space="PSUM"))

        for _rep in range(reps):
            _kernel_body(nc, tc, mybir, const, big, fe, ps, d, scal)

    nc.compile()
    return nc


def _kernel_body(nc, tc, mybir, const, big, fe, ps, d, scal):
    f32 = mybir.dt.float32
    bf = mybir.dt.bfloat16
    f8 = mybir.dt.float8e4
    AX = mybir.AxisListType.X
    MAX = mybir.AluOpType.max
    ADD = mybir.AluOpType.add
    EXP = mybir.ActivationFunctionType.Exp
    LN = mybir.ActivationFunctionType.Ln
    IDENT = mybir.ActivationFunctionType.Identity
    DR = mybir.MatmulPerfMode.DoubleRow

    # ---- constants / weights into SBUF ----
    # zmask[:, h, c] = (c==h)/32: head h's softmax colsum lands on psum row h
    zmask = const.tile([128, 8, 8], bf, name="zmask", tag="zmask")
    nc.vector.memset(zmask[:], 0.0)
    for h in range(8):
        nc.vector.memset(zmask[:, h, h:h + 1], 1.0 / S_OT)
    # DMA issue order matches first-use order: the front-end's first
    # matmuls need wq18+befT+txq; everything else can land later.
    w = {}
    w["wq18"] = const.tile([128, KT, D], f8, name="wq18_sb", tag="wq18_sb")
    nc.sync.dma_start(out=w["wq18"][:], in_=d["wq18"][:, :, :])
    befT = big.tile([128, KT, NQ], f8, name="bef8", tag="bef8")
    nc.sync.dma_start(out=befT[:], in_=d["befT"][:, :, :])
    txq = const.tile([128, KT * AL], f32, name="txq_sb", tag="txq_sb")
    nc.sync.dma_start(out=txq[:], in_=d["txq"][:, :])
    w["wk18"] = const.tile([128, KT, D], f8, name="wk18_sb", tag="wk18_sb")
    nc.sync.dma_start(out=w["wk18"][:], in_=d["wk18"][:, :, :])
    txk = const.tile([128, KT * AL], f32, name="txk_sb", tag="txk_sb")
    nc.sync.dma_start(out=txk[:], in_=d["txk"][:, :])
    w["wv18"] = const.tile([128, KT, D], f8, name="wv18_sb", tag="wv18_sb")
    nc.sync.dma_start(out=w["wv18"][:], in_=d["wv18"][:, :, :])
    txv = const.tile([8, AL * D], bf, name="txv_sb", tag="txv_sb")
    nc.sync.dma_start(out=txv[:], in_=d["txv"][:, :])
    hsel = const.tile([8, KT * 128], bf, name="hsel_sb", tag="hsel_sb")
    nc.sync.dma_start(out=hsel[:], in_=d["hsel"][:, :])
    hsel = hsel.rearrange("p (k c) -> p k c", k=KT)
    w["wqlw8"] = const.tile([128, KT, RK], f8, name="wqlw8_sb", tag="wqlw8_sb")
    nc.sync.dma_start(out=w["wqlw8"][:], in_=d["wqlw8"][:, :, :])
    bqlw = const.tile([128, RT], f32, name="bqlw_sb", tag="bqlw_sb")
    nc.sync.dma_start(out=bqlw[:], in_=d["bqlw"][:, :])
    amask = const.tile([128, TQ * AL], bf, name="amask_sb", tag="amask_sb")
    nc.sync.dma_start(out=amask[:], in_=d["amask"][:, :])
    aft = big.tile([128, RT, NKEY], f8, name="aft8", tag="aft8")
    for c0 in range(0, NKEY, NKEY // 4):
        nc.sync.dma_start(out=aft[:, :, c0:c0 + NKEY // 4],
                          in_=d["aftT"][:, :, c0:c0 + NKEY // 4])

    qlwT = big.tile([128, RT, NQ], f8, name="qlwT8", tag="qlwT8")

    def proj(dst, dst_col, src, src_col, wname, n, bias=None, scale=1.0,
             txt=None, txt_a=0, mout=KT):
        """dst[:, m, dst_col:+n] = fp8-DR W^T x src[:, :, src_col:+n];
        scale/bias (or per-batch txt bias) applied on the ACT copy."""
        for m in range(mout):
            p = ps.tile([128, 512], f32, name="p_proj", tag="a1", bufs=3)
            for j in range(KT // 2):
                nc.tensor.matmul(
                    p[:, 0:n], lhsT=w[wname][:, 2 * j:2 * j + 2,
                                            m * 128:(m + 1) * 128],
                    rhs=src[:, 2 * j:2 * j + 2, src_col:src_col + n],
                    start=(j == 0), stop=(j == KT // 2 - 1), perf_mode=DR)
            out_ap = dst[:, m, dst_col:dst_col + n]
            if txt is not None:
                for ab in range(n // LV):
                    a = txt_a + ab
                    nc.scalar.activation(
                        out_ap[:, ab * LV:(ab + 1) * LV],
                        p[:, ab * LV:(ab + 1) * LV], IDENT, scale=scale,
                        bias=txt[:, a * KT + m: a * KT + m + 1])
            elif bias is not None:
                nc.scalar.activation(out_ap, p[:, 0:n], IDENT, scale=scale,
                                     bias=bias[:, m:m + 1])
            else:
                nc.scalar.activation(out_ap, p[:, 0:n], IDENT, scale=scale)

    # ================= front-end (per apair) =================
    def fe_apair(apair):
        pc = apair * W2

        qt = fe.tile([128, KT, W2P], f8, name="qt", tag="qt")
        kt = fe.tile([128, KT, W2P], f8, name="kt", tag="kt")
        proj(qt, 0, befT, pc, "wq18", W2, scale=scal["q"], txt=txq,
             txt_a=apair * 2)
        yield
        proj(kt, 0, befT, pc, "wk18", W2, scale=scal["k"], txt=txk,
             txt_a=apair * 2)
        yield

        ot = fe.tile([128, KT, W2P], f8, name="ot", tag="ot")
        for ab in range(2):
            a = apair * 2 + ab
            ac = ab * LV
            # v position-major (196, 512) as two row tiles (bf16), no bias
            vpos = []
            for lt, (l0, ln) in enumerate(LSPLIT):
                p5 = ps.tile([128, 512], f32, name="p_vpos", tag="a1", bufs=3)
                for j in range(KT // 2):
                    nc.tensor.matmul(
                        p5[0:ln, :],
                        lhsT=befT[:, 2 * j:2 * j + 2, pc + ac + l0:pc + ac + l0 + ln],
                        rhs=w["wv18"][:, 2 * j:2 * j + 2, :],
                        start=(j == 0), stop=(j == KT // 2 - 1), perf_mode=DR)
                t = fe.tile([ln, D], bf, name=f"vpos_{lt}", tag=f"vpos_{lt}")
                nc.scalar.activation(t[:], p5[0:ln, :], IDENT,
                                     scale=scal["v"])
                vpos.append(t)
            yield

            # scores + exp for all heads; softmax colsums into one tile.
            # pzs sits in a g3 slot so the psc rotation through a1 can't
            # block on its long lifetime.
            eT = {}
            pzs = ps.tile([8, LV], f32, name="pzs", tag="pzs", bufs=1)
            for kt2 in range(KT):
                for hh in range(2):
                    h = kt2 * 2 + hh
                    off = 64 * hh
                    for mt, (m0, mn) in enumerate(LSPLIT):
                        psc = ps.tile([128, LV], f32, name="p_sc", tag="a1", bufs=3)
                        nc.tensor.matmul(
                            psc[0:mn, :],
                            lhsT=kt[off:off + 64, kt2, ac + m0:ac + m0 + mn],
                            rhs=qt[off:off + 64, kt2, ac:ac + LV],
                            start=True, stop=True)
                        e = fe.tile([mn, LV], bf, name=f"eT_{h}_{mt}",
                                    tag=f"eT_{h}_{mt}")
                        nc.scalar.activation(e[:], psc[0:mn, :], EXP,
                                             scale=0.125)
                        eT[(h, mt)] = e
                        nc.tensor.matmul(pzs[:], lhsT=zmask[0:mn, h, :],
                                         rhs=e[:],
                                         start=(h == 0 and mt == 0),
                                         stop=(h == 7 and mt == 1))
                yield
            # batched softmax normalizers: z row per head
            zrow = fe.tile([8, LV], bf, name="zrow", tag="zrow")
            nc.vector.tensor_copy(zrow[:], pzs[:])
            rz32 = fe.tile([8, LV], f32, name="rz32", tag="rz32")
            nc.vector.reciprocal_approx_fast(rz32[:], pzs[:])
            rzb = fe.tile([8, LV], bf, name="rzb", tag="rzb")
            nc.vector.tensor_copy(rzb[:], rz32[:])
            yield

            for kt2 in range(KT):
                pp = ps.tile([128, 2, 512], f32, name="pp", tag="g2")
                po = pp[:, 0, 0:LV]
                pzb = pp[:, 1, 0:LV]
                for hh in range(2):
                    h = kt2 * 2 + hh
                    off = 64 * hh
                    for mt, (m0, mn) in enumerate(LSPLIT):
                        nc.tensor.matmul(po[off:off + 64, :],
                                         lhsT=vpos[mt][:, h * 64:(h + 1) * 64],
                                         rhs=eT[(h, mt)][:], start=(mt == 0),
                                         stop=False)
                    # deferred conv1-text contribution: po += (32*txv) (x) z
                    nc.tensor.matmul(po[off:off + 64, :],
                                     lhsT=txv[0:8, (a * 8 + h) * 64:
                                              (a * 8 + h + 1) * 64],
                                     rhs=zrow[:], start=False, stop=True)
                nc.tensor.matmul(pzb[:], lhsT=hsel[0:8, kt2, :],
                                 rhs=rzb[:], start=True, stop=True)
                # tensor ops may read only ONE psum operand: stage pzb in SBUF
                zb = fe.tile([128, LV], bf, name="zb", tag="zb")
                nc.scalar.copy(zb[:], pzb)
                nc.vector.tensor_mul(ot[:, kt2, ac:ac + LV], po, zb[:])
                yield

        proj(qlwT, pc, ot, 0, "wqlw8", W2, bias=bqlw, scale=scal["qlw"],
             mout=RT)
        yield

    # ================= logits =================
    # rm[q, t, b]: per-tile row maxes (t2v); zeroed once for the t=6 rows
    # past qn=16 that the output DMA reads but no reduce writes
    rm = big.tile([128, TQ, B], bf, name="rm", tag="rm")
    nc.vector.memset(rm[:], 0.0)
    etearly = big.tile([128, NBP, 3, W2], bf, name="etearly", tag="etearly")

    def logit_group(bp, ts, et):
        """matmuls for q-tiles ts x key-pair bp; t2v reduce + v2t exp."""
        ng = len(ts)
        pg = ps.tile([128, 2, 512], f32, name="pg", tag="g2")
        qn_max = 0
        for i, t in enumerate(ts):
            qn = min(128, NQ - t * 128)
            qn_max = max(qn_max, qn)
            nc.tensor.matmul(
                pg[0:qn, i, 0:W2],
                lhsT=qlwT[:, 0:RT, t * 128:t * 128 + qn],
                rhs=aft[:, 0:RT, bp * W2:(bp + 1) * W2],
                start=True, stop=True, perf_mode=DR)
        # t2v: exact row max over each batch's 196 key columns
        nc.vector.tensor_reduce(
            rm[0:qn_max, ts[0]:ts[0] + ng, 2 * bp:2 * bp + 2],
            pg[0:qn_max, 0:ng, 0:W2].rearrange(
                "p g (two m) -> p g two m", two=2),
            axis=AX, op=MAX)
        # v2t: exp(beta x) for the partition-axis LSE
        nc.scalar.activation(et[0:qn_max, 0:ng, :], pg[0:qn_max, 0:ng, 0:W2],
                             EXP, scale=scal["beta"])

    def colsums(bp, zacc, et, ts, start, stop):
        for i, t in enumerate(ts):
            qn = min(128, NQ - t * 128)
            nc.tensor.matmul(zacc[:], lhsT=amask[0:qn, t * AL:(t + 1) * AL],
                             rhs=et[0:qn, i, :], start=start and i == 0,
                             stop=stop and i == len(ts) - 1)

    def logits_pass1():
        """t-tiles 0..2 for all bp (ready after apair 0): hold exp tiles."""
        for bp in range(NBP):
            logit_group(bp, [0, 1], etearly[:, bp, 0:2])
            yield
            logit_group(bp, [2], etearly[:, bp, 2:3])
            yield

    def logits_pass2():
        for bp in range(NBP):
            zacc = ps.tile([AL, W2], f32, name="zacc", tag="a1", bufs=3)
            colsums(bp, zacc, etearly[:, bp], [0, 1, 2], True, False)
            et2 = fe.tile([128, 2, W2], bf, name="et2", tag="et2")
            logit_group(bp, [3, 4], et2)
            colsums(bp, zacc, et2, [3, 4], False, False)
            et3 = fe.tile([128, 2, W2], bf, name="et3", tag="et3")
            logit_group(bp, [5], et3[:, 0:1])
            colsums(bp, zacc, et3[:, 0:1], [5], False, False)
            logit_group(bp, [6], et3[:, 1:2])
            colsums(bp, zacc, et3[:, 1:2], [6], False, True)
            # ln + sum over m run on the host (avoids Exp<->Ln ACT-table
            # thrash): ship the tiny (AL, 392) colsum tile out via SBUF
            zn = fe.tile([AL, W2], f32, name="zn", tag="zn")
            nc.scalar.copy(zn[:], zacc[:])
            nc.sync.dma_start(out=d["out2"][:, bp * W2:(bp + 1) * W2],
                              in_=zn[:])
            yield

    # ================= schedule =================
    for _ in fe_apair(0):
        pass
    # interleave apair-1 front-end with pass-1 logits (t 0..2 need only
    # apair-0's qlw rows)
    g1 = logits_pass1()
    gfe = fe_apair(1)
    done1 = done2 = False
    while not (done1 and done2):
        if not done2:
            done2 = next(gfe, "END") == "END"
        if not done1:
            done1 = next(g1, "END") == "END"
    for _ in logits_pass2():
        pass

    # ---- t2v: ship the per-tile row maxes; batch attribution on host
    nc.sync.dma_start(out=d["outrm"][:, :],
                      in_=rm.rearrange("p t b -> p (t b)"))


def get_program(scal, reps=1):
    key = ("nc", reps, tuple(sorted(scal.items())))
    if key not in _CACHE:
        _CACHE[key] = _build_program(scal, reps)
    return _CACHE[key]


def _to3d(mat512, cols, dtype, rows=D):
    """(rows, cols) feature-major -> (128, rows//128, cols) k-tile-major."""
    return np.ascontiguousarray(
        np.asarray(mat512, np.float32).reshape(rows // 128, 128, cols)
        .transpose(1, 0, 2)).astype(dtype)


def _pcol(vec, scale, rows=D):
    """(rows,) -> (128, rows//128) partition-major f32."""
    return np.ascontiguousarray(
        (np.asarray(vec, np.float32) * scale).reshape(rows // 128, 128).T
    ).astype(np.float32)


def _host_forward(bef, txtc, Wq1, Wk1, Wv1, txq, txk, txv, Wqlw, bw):
    """f32 reference front-end, used only to calibrate fp8/exp scales."""
    q = np.einsum("bld,od->blo", bef, Wq1) + txq[:, None, :]
    k = np.einsum("bld,od->blo", bef, Wk1) + txk[:, None, :]
    v = np.einsum("bld,od->blo", bef, Wv1) + txv[:, None, :]
    DH = D // H
    th = lambda t: t.reshape(B, LV, H, DH).transpose(0, 2, 1, 3)
    qh, kh, vh = th(q), th(k), th(v)
    sc = np.einsum("bhld,bhmd->bhlm", qh, kh)
    e = np.exp(sc * 0.125)
    at = e / e.sum(-1, keepdims=True)
    o = np.einsum("bhlm,bhmd->bhld", at, vh)
    ot = o.transpose(0, 2, 1, 3).reshape(B, LV, D)
    qlw = np.einsum("bld,od->blo", ot, Wqlw) + bw[None, None, :]
    return ot, qlw


def make_in_maps(bef_feat, sent_feat, aft_feat, masks,
                 conv1_w, conv1_b, in_proj_w, out_proj_w, conv2_w, conv2_b,
                 q_w, q_b, k_w, k_b, logit_scale):
    bef_feat = np.asarray(bef_feat, np.float32)
    sent_feat = np.asarray(sent_feat, np.float32)
    aft_feat = np.asarray(aft_feat, np.float32)
    masks = np.asarray(masks, np.float32)
    conv1_w = np.asarray(conv1_w, np.float32)
    in_proj_w = np.asarray(in_proj_w, np.float32)
    Wo = np.asarray(out_proj_w, np.float32)
    Wc2 = np.asarray(conv2_w, np.float32)
    Wql = np.asarray(q_w, np.float32)
    Wkl = np.asarray(k_w, np.float32)

    sent_mean = (sent_feat * masks[:, :, None]).mean(axis=1)       # (B, D)
    txtc = sent_mean @ conv1_w[:, D:].T + np.asarray(conv1_b, np.float32)
    W1a = conv1_w[:, :D]
    Wq, Wk, Wv = np.split(in_proj_w, 3, axis=0)

    # fused weights / biases
    Wq1, Wk1, Wv1 = Wq @ W1a, Wk @ W1a, Wv @ W1a
    txq, txk, txv = txtc @ Wq.T, txtc @ Wk.T, txtc @ Wv.T          # (B, D)
    Wql3 = Wql @ Wc2 @ Wo
    bias3 = np.asarray(conv2_b, np.float32) @ Wql.T + np.asarray(q_b, np.float32)
    Wqlw = Wkl @ Wql3
    biasw = bias3 @ Wkl.T
    if np.abs(np.asarray(k_b, np.float32)).max() > 0:
        raise NotImplementedError("nonzero k_b not supported by this kernel")

    # low-rank factorization of the fused logits weight (augmented with
    # the bias column so it is carried exactly):
    #   [Wqlw | biasw] ~ A @ Bm,  A (D, RK), Bm (RK, D+1)
    # logits = (Bm[:, :D].ot + Bm[:, D]) . (A^T aft) -- the key-side
    # projection A^T aft is precomputed on the host for free.
    M = np.concatenate([Wqlw, biasw[:, None]], axis=1)
    U, sv, Vt = np.linalg.svd(M, full_matrices=False)
    A = U[:, :RK] * np.sqrt(sv[:RK])[None, :]
    Bm = np.sqrt(sv[:RK])[:, None] * Vt[:RK]
    Wqlw_r = Bm[:, :D]                 # (RK, D): device projection weight
    biasw_r = Bm[:, D]                 # (RK,)
    aft_r = aft_feat @ A               # (B, LV, RK) host key features

    # scale calibration from a host f32 forward
    ot_f, qlw_f = _host_forward(bef_feat, txtc, Wq1, Wk1, Wv1,
                                txq, txk, txv, Wqlw, biasw)
    qlwr_f = np.einsum("bld,rd->blr", ot_f, Wqlw_r) + biasw_r[None, None, :]
    SW = {}
    for nm, wm in [("q", Wq1), ("k", Wk1), ("v", Wv1), ("w", Wqlw_r)]:
        SW[nm] = 200.0 / max(np.abs(wm).max(), 1e-30)
    SL = 100.0 / max(np.abs(qlwr_f).max(), 1e-30)
    SA = 200.0 / max(np.abs(aft_r).max(), 1e-30)
    # beta: norm bound on |X_psum| = |SL*qlw_r . SA*aft_r|
    bnd = (np.linalg.norm(qlwr_f * SL, axis=-1).max()
           * np.linalg.norm(aft_r * SA, axis=-1).max())
    beta = 80.0 / bnd
    scal = {
        "q": 1.0 / SW["q"],               # ACT copy scale for qt/kt
        "v": 1.0 / SW["v"],
        "qlw": SL / (S_OT * SW["w"]),
        "beta": float(beta),
        "os_t2v": float(1.0 / (LV * SL * SA)),
        "os_v2t": float(1.0 / (LV * SL * SA * beta)),
    }

    aftT = _to3d((aft_r * SA).transpose(2, 0, 1).reshape(RK, NKEY),
                 NKEY, F8, rows=RK)
    amask = np.zeros((128, TQ * AL), np.float32)
    for t in range(TQ):
        for r in range(min(128, NQ - t * 128)):
            amask[r, t * AL + (t * 128 + r) // LV] = 1.0

    wmats = {
        "wq18": _to3d(Wq1.T * SW["q"], D, F8),
        "wk18": _to3d(Wk1.T * SW["k"], D, F8),
        "wv18": _to3d(Wv1.T * SW["v"], D, F8),
        "wqlw8": _to3d(Wqlw_r.T * SW["w"], RK, F8),
    }
    # kt is produced with scale["q"] too; fold the k-weight scale difference
    # into the ACT copy: we used SW["q"] for both ACT scales, so scale the
    # k weight so psum/SW_q is correct: wk18 holds Wk1*SW_k; ACT scale must
    # be 1/SW_k.  Keep separate scale entries instead.
    scal["k"] = 1.0 / SW["k"]

    bqlw = _pcol(biasw_r, SL, rows=RK)

    in_maps = []
    for c in range(NCORES):
        sl = slice(c * AL, (c + 1) * AL)
        befT = _to3d(bef_feat[sl].transpose(2, 0, 1).reshape(D, NQ), NQ, F8)
        txq_t = np.zeros((128, KT * AL), np.float32)
        txk_t = np.zeros((128, KT * AL), np.float32)
        for a in range(AL):
            txq_t[:, a * KT:(a + 1) * KT] = txq[c * AL + a].reshape(KT, 128).T
            txk_t[:, a * KT:(a + 1) * KT] = txk[c * AL + a].reshape(KT, 128).T
        txv_t = np.zeros((8, AL * D), np.float32)
        for a in range(AL):
            for h in range(H):
                txv_t[h, (a * 8 + h) * 64:(a * 8 + h + 1) * 64] = \
                    S_OT * txv[c * AL + a, h * 64:(h + 1) * 64]
        txv_t = txv_t.astype(BF16)
        hsel = np.zeros((8, KT, 128), np.float32)
        for kt2 in range(KT):
            hsel[2 * kt2, kt2, 0:64] = 1.0
            hsel[2 * kt2 + 1, kt2, 64:128] = 1.0
        m = {"befT": befT, "aftT": aftT, "txq": txq_t, "txk": txk_t,
             "txv": txv_t, "bqlw": bqlw, "amask": amask.astype(BF16),
             "hsel": hsel.reshape(8, KT * 128).astype(BF16)}
        m.update(wmats)
        in_maps.append(m)
    return in_maps, scal


def rows_from_outputs(res, scal):
    """One core's {out2, outrm} -> (t2v rows (AL,B), v2t rows (AL,B))."""
    outrm = np.asarray(res["outrm"], np.float64).reshape(128, TQ, B)
    out2 = np.asarray(res["out2"], np.float64).reshape(AL, NBP, 2, LV)
    t2v = np.zeros((AL, B))
    for t in range(TQ):
        qn = min(128, NQ - t * 128)
        for r in range(qn):
            t2v[(t * 128 + r) // LV] += outrm[r, t]
    t2v *= scal["os_t2v"]
    v2t = np.log(out2).sum(axis=3).reshape(AL, B) * scal["os_v2t"]
    return t2v, v2t


def finish(results, scal, logit_scale):
    """results: list of 8 per-core {out2, outrm} dicts -> scalar loss."""
    t2v = np.zeros((B, B), np.float64)
    v2t = np.zeros((B, B), np.float64)
    for c in range(NCORES):
        tr, vr = rows_from_outputs(results[c], scal)
        t2v[c * AL:(c + 1) * AL, :] = tr
        v2t[c * AL:(c + 1) * AL, :] = vr
    S = 0.5 * (t2v + v2t) * np.exp(np.float64(np.asarray(logit_scale)))

    def ce(m):
        lse = np.log(np.sum(np.exp(m - m.max(axis=1, keepdims=True)), axis=1)) \
            + m.max(axis=1)
        return -np.mean(np.diag(m) - lse)

    return np.float32(0.5 * (ce(S) + ce(S.T)))


def kernel(**inputs):
    from concourse.bass_utils import run_bass_kernel_spmd

    in_maps, scal = make_in_maps(**inputs)
    nc = get_program(scal)
    res = run_bass_kernel_spmd(nc, in_maps, core_ids=list(range(NCORES)))
    return finish(res.results, scal, inputs["logit_scale"])



# revision 24
# speedup vs baseline: 1.2817x; 1.2817x over previous
"""Trainium2 Bass kernel for the cross-batch retrieval contrastive loss.

Reference semantics per batch b:
  sent_mean = mean(sent_feat * masks)                      (host)
  v1   = conv1([bef^T; broadcast sent_mean])               -> (196, 512)
  MHA over 196 positions, out_proj                         -> (196, 512)
  mod  = conv2(o); ql = mod @ q_w^T + q_b                  -> (196, 512)
  kl   = aft @ k_w^T + k_b                                 -> (196, 512)
  logits[a,b,l,m] = ql[a,l,:] . kl[b,m,:]
  t2v[a,b] = mean_l max_m ; v2t[a,b] = mean_m max_l
  loss = symmetric InfoNCE on S = 0.5*(t2v+v2t)*exp(logit_scale)   (host)

Key algebraic restructurings (all host-side weight folding):
  - conv1 folds into the qkv projections: q = (Wq@W1a).bef + Wq.txt(a),
    so the v1 intermediate never exists on device.  The per-batch text
    contribution is an ACT-copy bias for q/k; for v it is deferred to
    the attention output via po += (32*txv) (x) z, because
    (po + c(x)z) * (1/z) = po/z + c.
  - out_proj, conv2, q_w and k_w^T all fold into ONE weight:
      logits = ql . (Wkl.aft) = ((Wkl@Wql@Wc2@Wo).ot) . aft
    so kl is never materialized: `aft` (already fp8 in SBUF) is the
    logits moving operand directly, and the post-attention front-end is
    a single projection ot -> qlw.
  - t2v: exact row-max on DVE over 3-bank PSUM groups (G=3 batching
    amortizes the PSUM-access + seq overhead per reduce).
  - v2t: log-sum-exp over the partition axis: ACT exp (scale=beta) of
    the same PSUM tiles -> PE colsum with the amask indicator
    (attributes q-rows to their batch) -> one ACT ln per key-pair ->
    small DVE add-reduce.  beta is host-calibrated from a norm bound so
    max |beta*X| ~ 13 and the LSE error is ~1e-5 relative (the fp8
    quantization error of ~5e-4 dominates).
  - softmax normalizers: all 8 heads' colsums land in one (8,196) PSUM
    tile; ONE reciprocal_approx_fast per batch replaces 32 full-precision
    DVE reciprocals (1.4us each) on the critical path.

Sharding: data-parallel over the query-batch axis 'a' (4 batches/core x
8 cores); aft/logits key side replicated. The final 32x32 InfoNCE runs
on the host in float64 (tiny).
"""

import numpy as np
import ml_dtypes

B, LV, LT, D, H = 32, 196, 40, 512, 8
NCORES = 8
AL = B // NCORES          # query batches per core
KT = D // 128             # 128-row feature tiles per 512-dim tensor
LSPLIT = [(0, 128), (128, 68)]   # 196 = 128 + 68
NQ = AL * LV              # 784 query position-rows per core
NKEY = B * LV             # 6272 key position-rows
TQ = (NQ + 127) // 128    # 7 stationary tiles over query rows
NBP = B // 2              # 16 key-batch pairs
W2 = 2 * LV               # batch-pair moving width
W2P = 400                 # fe fp8 tile stride (16B-aligned for DoubleRow)
RK = 256                  # low-rank factorization of the fused logits weight
RT = RK // 128            # its 128-row tiles
S_OT = 32.0               # ot fp8 scale (from the 1/32 colsum)
BF16 = ml_dtypes.bfloat16
F8 = ml_dtypes.float8_e4m3fn

_CACHE = {}


def _build_program(scal, reps=1):
    from contextlib import ExitStack
    import concourse.bacc as bacc
    import concourse.tile as tile
    from concourse import mybir

    f32 = mybir.dt.float32
    bf = mybir.dt.bfloat16
    f8 = mybir.dt.float8e4

    nc = bacc.Bacc("TRN2", target_bir_lowering=False, debug=False,
                   num_devices=NCORES)

    d = {
        "befT": nc.dram_tensor("befT", [128, KT, NQ], f8,
                               kind="ExternalInput").ap(),
        "aftT": nc.dram_tensor("aftT", [128, RT, NKEY], f8,
                               kind="ExternalInput").ap(),
        # per-batch per-partition biases for the fused q/k projections
        "txq": nc.dram_tensor("txq", [128, KT * AL], f32,
                              kind="ExternalInput").ap(),
        "txk": nc.dram_tensor("txk", [128, KT * AL], f32,
                              kind="ExternalInput").ap(),
        # txv selector-stationary: txv[r, (a*8+h)*64+c] = (r==h)*32*txv[a,h*64+c]
        "txv": nc.dram_tensor("txv", [8, AL * D], bf,
                              kind="ExternalInput").ap(),
        "bqlw": nc.dram_tensor("bqlw", [128, RT], f32,
                               kind="ExternalInput").ap(),
        "amask": nc.dram_tensor("amask", [128, TQ * AL], bf,
                                kind="ExternalInput").ap(),
        "hsel": nc.dram_tensor("hsel", [8, KT * 128], bf,
                               kind="ExternalInput").ap(),
        "xt": nc.dram_tensor("xt", [128, NBP * TQ * W2], bf,
                             kind="ExternalOutput").ap(),
    }
    for n in ["wq18", "wk18", "wv18"]:
        d[n] = nc.dram_tensor(n, [128, KT, D], f8, kind="ExternalInput").ap()
    d["wqlw8"] = nc.dram_tensor("wqlw8", [128, KT, RK], f8,
                                kind="ExternalInput").ap()

    with tile.TileContext(nc) as tc, ExitStack() as ctx:
        const = ctx.enter_context(tc.tile_pool(name="const", bufs=1))
        big = ctx.enter_context(tc.tile_pool(name="big", bufs=1))
        fe = ctx.enter_context(tc.tile_pool(name="fe", bufs=2))
        # PSUM budget (8 banks): a1 3x1 + pzs 1x1 + g2 2x2
        ps = ctx.enter_context(tc.tile_pool(name="ps", bufs=2, space="PSUM"))

        for _rep in range(reps):
            _kernel_body(nc, tc, mybir, const, big, fe, ps, d, scal)

    nc.compile()
    return nc


def _kernel_body(nc, tc, mybir, const, big, fe, ps, d, scal):
    f32 = mybir.dt.float32
    bf = mybir.dt.bfloat16
    f8 = mybir.dt.float8e4
    AX = mybir.AxisListType.X
    MAX = mybir.AluOpType.max
    ADD = mybir.AluOpType.add
    EXP = mybir.ActivationFunctionType.Exp
    LN = mybir.ActivationFunctionType.Ln
    IDENT = mybir.ActivationFunctionType.Identity
    DR = mybir.MatmulPerfMode.DoubleRow

    # ---- constants / weights into SBUF ----
    # zmask[:, h, c] = (c==h)/32: head h's softmax colsum lands on psum row h
    zmask = const.tile([128, 8, 8], bf, name="zmask", tag="zmask")
    nc.vector.memset(zmask[:], 0.0)
    for h in range(8):
        nc.vector.memset(zmask[:, h, h:h + 1], 1.0 / S_OT)
    # DMA issue order matches first-use order: the front-end's first
    # matmuls need wq18+befT+txq; everything else can land later.
    w = {}
    w["wq18"] = const.tile([128, KT, D], f8, name="wq18_sb", tag="wq18_sb")
    nc.sync.dma_start(out=w["wq18"][:], in_=d["wq18"][:, :, :])
    befT = big.tile([128, KT, NQ], f8, name="bef8", tag="bef8")
    nc.sync.dma_start(out=befT[:], in_=d["befT"][:, :, :])
    txq = const.tile([128, KT * AL], f32, name="txq_sb", tag="txq_sb")
    nc.sync.dma_start(out=txq[:], in_=d["txq"][:, :])
    w["wk18"] = const.tile([128, KT, D], f8, name="wk18_sb", tag="wk18_sb")
    nc.sync.dma_start(out=w["wk18"][:], in_=d["wk18"][:, :, :])
    txk = const.tile([128, KT * AL], f32, name="txk_sb", tag="txk_sb")
    nc.sync.dma_start(out=txk[:], in_=d["txk"][:, :])
    w["wv18"] = const.tile([128, KT, D], f8, name="wv18_sb", tag="wv18_sb")
    nc.sync.dma_start(out=w["wv18"][:], in_=d["wv18"][:, :, :])
    txv = const.tile([8, AL * D], bf, name="txv_sb", tag="txv_sb")
    nc.sync.dma_start(out=txv[:], in_=d["txv"][:, :])
    hsel = const.tile([8, KT * 128], bf, name="hsel_sb", tag="hsel_sb")
    nc.sync.dma_start(out=hsel[:], in_=d["hsel"][:, :])
    hsel = hsel.rearrange("p (k c) -> p k c", k=KT)
    w["wqlw8"] = const.tile([128, KT, RK], f8, name="wqlw8_sb", tag="wqlw8_sb")
    nc.sync.dma_start(out=w["wqlw8"][:], in_=d["wqlw8"][:, :, :])
    bqlw = const.tile([128, RT], f32, name="bqlw_sb", tag="bqlw_sb")
    nc.sync.dma_start(out=bqlw[:], in_=d["bqlw"][:, :])
    amask = const.tile([128, TQ * AL], bf, name="amask_sb", tag="amask_sb")
    nc.sync.dma_start(out=amask[:], in_=d["amask"][:, :])
    aft = big.tile([128, RT, NKEY], f8, name="aft8", tag="aft8")
    for c0 in range(0, NKEY, NKEY // 4):
        nc.sync.dma_start(out=aft[:, :, c0:c0 + NKEY // 4],
                          in_=d["aftT"][:, :, c0:c0 + NKEY // 4])

    qlwT = big.tile([128, RT, NQ], f8, name="qlwT8", tag="qlwT8")

    def proj(dst, dst_col, src, src_col, wname, n, bias=None, scale=1.0,
             txt=None, txt_a=0, mout=KT):
        """dst[:, m, dst_col:+n] = fp8-DR W^T x src[:, :, src_col:+n];
        scale/bias (or per-batch txt bias) applied on the ACT copy."""
        for m in range(mout):
            p = ps.tile([128, 512], f32, name="p_proj", tag="a1", bufs=3)
            for j in range(KT // 2):
                nc.tensor.matmul(
                    p[:, 0:n], lhsT=w[wname][:, 2 * j:2 * j + 2,
                                            m * 128:(m + 1) * 128],
                    rhs=src[:, 2 * j:2 * j + 2, src_col:src_col + n],
                    start=(j == 0), stop=(j == KT // 2 - 1), perf_mode=DR)
            out_ap = dst[:, m, dst_col:dst_col + n]
            if txt is not None:
                for ab in range(n // LV):
                    a = txt_a + ab
                    nc.scalar.activation(
                        out_ap[:, ab * LV:(ab + 1) * LV],
                        p[:, ab * LV:(ab + 1) * LV], IDENT, scale=scale,
                        bias=txt[:, a * KT + m: a * KT + m + 1])
            elif bias is not None:
                nc.scalar.activation(out_ap, p[:, 0:n], IDENT, scale=scale,
                                     bias=bias[:, m:m + 1])
            else:
                nc.scalar.activation(out_ap, p[:, 0:n], IDENT, scale=scale)

    # ================= front-end (per apair) =================
    def fe_apair(apair):
        pc = apair * W2

        qt = fe.tile([128, KT, W2P], f8, name="qt", tag="qt")
        kt = fe.tile([128, KT, W2P], f8, name="kt", tag="kt")
        proj(qt, 0, befT, pc, "wq18", W2, scale=scal["q"], txt=txq,
             txt_a=apair * 2)
        yield
        proj(kt, 0, befT, pc, "wk18", W2, scale=scal["k"], txt=txk,
             txt_a=apair * 2)
        yield

        ot = fe.tile([128, KT, W2P], f8, name="ot", tag="ot")
        for ab in range(2):
            a = apair * 2 + ab
            ac = ab * LV
            # v position-major (196, 512) as two row tiles (bf16), no bias
            vpos = []
            for lt, (l0, ln) in enumerate(LSPLIT):
                p5 = ps.tile([128, 512], f32, name="p_vpos", tag="a1", bufs=3)
                for j in range(KT // 2):
                    nc.tensor.matmul(
                        p5[0:ln, :],
                        lhsT=befT[:, 2 * j:2 * j + 2, pc + ac + l0:pc + ac + l0 + ln],
                        rhs=w["wv18"][:, 2 * j:2 * j + 2, :],
                        start=(j == 0), stop=(j == KT // 2 - 1), perf_mode=DR)
                t = fe.tile([ln, D], bf, name=f"vpos_{lt}", tag=f"vpos_{lt}")
                nc.scalar.activation(t[:], p5[0:ln, :], IDENT,
                                     scale=scal["v"])
                vpos.append(t)
            yield

            # scores + exp for all heads; softmax colsums into one tile.
            # pzs sits in a g3 slot so the psc rotation through a1 can't
            # block on its long lifetime.
            eT = {}
            pzs = ps.tile([8, LV], f32, name="pzs", tag="pzs", bufs=1)
            for kt2 in range(KT):
                for hh in range(2):
                    h = kt2 * 2 + hh
                    off = 64 * hh
                    for mt, (m0, mn) in enumerate(LSPLIT):
                        psc = ps.tile([128, LV], f32, name="p_sc", tag="a1", bufs=3)
                        nc.tensor.matmul(
                            psc[0:mn, :],
                            lhsT=kt[off:off + 64, kt2, ac + m0:ac + m0 + mn],
                            rhs=qt[off:off + 64, kt2, ac:ac + LV],
                            start=True, stop=True)
                        e = fe.tile([mn, LV], bf, name=f"eT_{h}_{mt}",
                                    tag=f"eT_{h}_{mt}")
                        nc.scalar.activation(e[:], psc[0:mn, :], EXP,
                                             scale=0.125)
                        eT[(h, mt)] = e
                        nc.tensor.matmul(pzs[:], lhsT=zmask[0:mn, h, :],
                                         rhs=e[:],
                                         start=(h == 0 and mt == 0),
                                         stop=(h == 7 and mt == 1))
                yield
            # batched softmax normalizers: z row per head
            zrow = fe.tile([8, LV], bf, name="zrow", tag="zrow")
            nc.vector.tensor_copy(zrow[:], pzs[:])
            rz32 = fe.tile([8, LV], f32, name="rz32", tag="rz32")
            nc.vector.reciprocal_approx_fast(rz32[:], pzs[:])
            rzb = fe.tile([8, LV], bf, name="rzb", tag="rzb")
            nc.vector.tensor_copy(rzb[:], rz32[:])
            yield

            for kt2 in range(KT):
                pp = ps.tile([128, 2, 512], f32, name="pp", tag="g2")
                po = pp[:, 0, 0:LV]
                pzb = pp[:, 1, 0:LV]
                for hh in range(2):
                    h = kt2 * 2 + hh
                    off = 64 * hh
                    for mt, (m0, mn) in enumerate(LSPLIT):
                        nc.tensor.matmul(po[off:off + 64, :],
                                         lhsT=vpos[mt][:, h * 64:(h + 1) * 64],
                                         rhs=eT[(h, mt)][:], start=(mt == 0),
                                         stop=False)
                    # deferred conv1-text contribution: po += (32*txv) (x) z
                    nc.tensor.matmul(po[off:off + 64, :],
                                     lhsT=txv[0:8, (a * 8 + h) * 64:
                                              (a * 8 + h + 1) * 64],
                                     rhs=zrow[:], start=False, stop=True)
                nc.tensor.matmul(pzb[:], lhsT=hsel[0:8, kt2, :],
                                 rhs=rzb[:], start=True, stop=True)
                # tensor ops may read only ONE psum operand: stage pzb in SBUF
                zb = fe.tile([128, LV], bf, name="zb", tag="zb")
                nc.scalar.copy(zb[:], pzb)
                nc.vector.tensor_mul(ot[:, kt2, ac:ac + LV], po, zb[:])
                yield

        proj(qlwT, pc, ot, 0, "wqlw8", W2, bias=bqlw, scale=scal["qlw"],
             mout=RT)
        yield

    # ================= logits: matmul + bf16 staging + DMA out ========
    def logits_unit(bp, ts, xtile, off):
        pg = ps.tile([128, 2, 512], f32, name="pg", tag="g2")
        for i, t in enumerate(ts):
            qn = min(128, NQ - t * 128)
            nc.tensor.matmul(
                pg[0:qn, i, 0:W2],
                lhsT=qlwT[:, 0:RT, t * 128:t * 128 + qn],
                rhs=aft[:, 0:RT, bp * W2:(bp + 1) * W2],
                start=True, stop=True, perf_mode=DR)
        ng = len(ts)
        nc.scalar.activation(xtile[:, off:off + ng, :], pg[:, 0:ng, 0:W2],
                             mybir.ActivationFunctionType.Copy)

    def xt_flush(bp, xtile, t0, nt):
        dst = d["xt"][:, (bp * TQ + t0) * W2:(bp * TQ + t0 + nt) * W2]
        nc.sync.dma_start(out=dst,
                          in_=xtile[:, 0:nt, :].rearrange("p a b -> p (a b)"))

    def logits_pass1():
        for bp in range(NBP):
            xa = fe.tile([128, 3, W2], bf, name="xa", tag="xa", bufs=3)
            logits_unit(bp, [0, 1], xa, 0)
            yield
            logits_unit(bp, [2], xa, 2)
            xt_flush(bp, xa, 0, 3)
            yield

    def logits_pass2():
        for bp in range(NBP):
            xb = fe.tile([128, 4, W2], bf, name="xb", tag="xb", bufs=3)
            logits_unit(bp, [3, 4], xb, 0)
            yield
            logits_unit(bp, [5, 6], xb, 2)
            xt_flush(bp, xb, 3, 4)
            yield

    # ================= schedule =================
    for _ in fe_apair(0):
        pass
    # interleave apair-1 front-end with pass-1 logits (t 0..2 need only
    # apair-0's qlw rows)
    g1 = logits_pass1()
    gfe = fe_apair(1)
    done1 = done2 = False
    while not (done1 and done2):
        if not done2:
            done2 = next(gfe, "END") == "END"
        if not done1:
            done1 = next(g1, "END") == "END"
    for _ in logits_pass2():
        pass




def get_program(scal, reps=1):
    key = ("nc", reps, tuple(sorted(scal.items())))
    if key not in _CACHE:
        _CACHE[key] = _build_program(scal, reps)
    return _CACHE[key]


def _to3d(mat512, cols, dtype, rows=D):
    """(rows, cols) feature-major -> (128, rows//128, cols) k-tile-major."""
    return np.ascontiguousarray(
        np.asarray(mat512, np.float32).reshape(rows // 128, 128, cols)
        .transpose(1, 0, 2)).astype(dtype)


def _pcol(vec, scale, rows=D):
    """(rows,) -> (128, rows//128) partition-major f32."""
    return np.ascontiguousarray(
        (np.asarray(vec, np.float32) * scale).reshape(rows // 128, 128).T
    ).astype(np.float32)


def _host_forward(bef, txtc, Wq1, Wk1, Wv1, txq, txk, txv, Wqlw, bw):
    """f32 reference front-end, used only to calibrate fp8/exp scales."""
    q = np.einsum("bld,od->blo", bef, Wq1) + txq[:, None, :]
    k = np.einsum("bld,od->blo", bef, Wk1) + txk[:, None, :]
    v = np.einsum("bld,od->blo", bef, Wv1) + txv[:, None, :]
    DH = D // H
    th = lambda t: t.reshape(B, LV, H, DH).transpose(0, 2, 1, 3)
    qh, kh, vh = th(q), th(k), th(v)
    sc = np.einsum("bhld,bhmd->bhlm", qh, kh)
    e = np.exp(sc * 0.125)
    at = e / e.sum(-1, keepdims=True)
    o = np.einsum("bhlm,bhmd->bhld", at, vh)
    ot = o.transpose(0, 2, 1, 3).reshape(B, LV, D)
    qlw = np.einsum("bld,od->blo", ot, Wqlw) + bw[None, None, :]
    return ot, qlw


def make_in_maps(bef_feat, sent_feat, aft_feat, masks,
                 conv1_w, conv1_b, in_proj_w, out_proj_w, conv2_w, conv2_b,
                 q_w, q_b, k_w, k_b, logit_scale):
    bef_feat = np.asarray(bef_feat, np.float32)
    sent_feat = np.asarray(sent_feat, np.float32)
    aft_feat = np.asarray(aft_feat, np.float32)
    masks = np.asarray(masks, np.float32)
    conv1_w = np.asarray(conv1_w, np.float32)
    in_proj_w = np.asarray(in_proj_w, np.float32)
    Wo = np.asarray(out_proj_w, np.float32)
    Wc2 = np.asarray(conv2_w, np.float32)
    Wql = np.asarray(q_w, np.float32)
    Wkl = np.asarray(k_w, np.float32)

    sent_mean = (sent_feat * masks[:, :, None]).mean(axis=1)       # (B, D)
    txtc = sent_mean @ conv1_w[:, D:].T + np.asarray(conv1_b, np.float32)
    W1a = conv1_w[:, :D]
    Wq, Wk, Wv = np.split(in_proj_w, 3, axis=0)

    # fused weights / biases
    Wq1, Wk1, Wv1 = Wq @ W1a, Wk @ W1a, Wv @ W1a
    txq, txk, txv = txtc @ Wq.T, txtc @ Wk.T, txtc @ Wv.T          # (B, D)
    Wql3 = Wql @ Wc2 @ Wo
    bias3 = np.asarray(conv2_b, np.float32) @ Wql.T + np.asarray(q_b, np.float32)
    Wqlw = Wkl @ Wql3
    biasw = bias3 @ Wkl.T
    if np.abs(np.asarray(k_b, np.float32)).max() > 0:
        raise NotImplementedError("nonzero k_b not supported by this kernel")

    # low-rank factorization of the fused logits weight (augmented with
    # the bias column so it is carried exactly):
    #   [Wqlw | biasw] ~ A @ Bm,  A (D, RK), Bm (RK, D+1)
    # logits = (Bm[:, :D].ot + Bm[:, D]) . (A^T aft) -- the key-side
    # projection A^T aft is precomputed on the host for free.
    M = np.concatenate([Wqlw, biasw[:, None]], axis=1)
    U, sv, Vt = np.linalg.svd(M, full_matrices=False)
    A = U[:, :RK] * np.sqrt(sv[:RK])[None, :]
    Bm = np.sqrt(sv[:RK])[:, None] * Vt[:RK]
    Wqlw_r = Bm[:, :D]                 # (RK, D): device projection weight
    biasw_r = Bm[:, D]                 # (RK,)
    aft_r = aft_feat @ A               # (B, LV, RK) host key features

    # scale calibration from a host f32 forward
    ot_f, qlw_f = _host_forward(bef_feat, txtc, Wq1, Wk1, Wv1,
                                txq, txk, txv, Wqlw, biasw)
    qlwr_f = np.einsum("bld,rd->blr", ot_f, Wqlw_r) + biasw_r[None, None, :]
    SW = {}
    for nm, wm in [("q", Wq1), ("k", Wk1), ("v", Wv1), ("w", Wqlw_r)]:
        SW[nm] = 200.0 / max(np.abs(wm).max(), 1e-30)
    SL = 100.0 / max(np.abs(qlwr_f).max(), 1e-30)
    SA = 200.0 / max(np.abs(aft_r).max(), 1e-30)
    # beta: norm bound on |X_psum| = |SL*qlw_r . SA*aft_r|
    bnd = (np.linalg.norm(qlwr_f * SL, axis=-1).max()
           * np.linalg.norm(aft_r * SA, axis=-1).max())
    beta = 80.0 / bnd
    scal = {
        "q": 1.0 / SW["q"],               # ACT copy scale for qt/kt
        "v": 1.0 / SW["v"],
        "qlw": SL / (S_OT * SW["w"]),
        "beta": float(beta),
        "os_t2v": float(1.0 / (LV * SL * SA)),
        "osx": float(1.0 / (SL * SA)),
        "os_v2t": float(1.0 / (LV * SL * SA * beta)),
    }

    aftT = _to3d((aft_r * SA).transpose(2, 0, 1).reshape(RK, NKEY),
                 NKEY, F8, rows=RK)
    amask = np.zeros((128, TQ * AL), np.float32)
    for t in range(TQ):
        for r in range(min(128, NQ - t * 128)):
            amask[r, t * AL + (t * 128 + r) // LV] = 1.0

    wmats = {
        "wq18": _to3d(Wq1.T * SW["q"], D, F8),
        "wk18": _to3d(Wk1.T * SW["k"], D, F8),
        "wv18": _to3d(Wv1.T * SW["v"], D, F8),
        "wqlw8": _to3d(Wqlw_r.T * SW["w"], RK, F8),
    }
    # kt is produced with scale["q"] too; fold the k-weight scale difference
    # into the ACT copy: we used SW["q"] for both ACT scales, so scale the
    # k weight so psum/SW_q is correct: wk18 holds Wk1*SW_k; ACT scale must
    # be 1/SW_k.  Keep separate scale entries instead.
    scal["k"] = 1.0 / SW["k"]

    bqlw = _pcol(biasw_r, SL, rows=RK)

    in_maps = []
    for c in range(NCORES):
        sl = slice(c * AL, (c + 1) * AL)
        befT = _to3d(bef_feat[sl].transpose(2, 0, 1).reshape(D, NQ), NQ, F8)
        txq_t = np.zeros((128, KT * AL), np.float32)
        txk_t = np.zeros((128, KT * AL), np.float32)
        for a in range(AL):
            txq_t[:, a * KT:(a + 1) * KT] = txq[c * AL + a].reshape(KT, 128).T
            txk_t[:, a * KT:(a + 1) * KT] = txk[c * AL + a].reshape(KT, 128).T
        txv_t = np.zeros((8, AL * D), np.float32)
        for a in range(AL):
            for h in range(H):
                txv_t[h, (a * 8 + h) * 64:(a * 8 + h + 1) * 64] = \
                    S_OT * txv[c * AL + a, h * 64:(h + 1) * 64]
        txv_t = txv_t.astype(BF16)
        hsel = np.zeros((8, KT, 128), np.float32)
        for kt2 in range(KT):
            hsel[2 * kt2, kt2, 0:64] = 1.0
            hsel[2 * kt2 + 1, kt2, 64:128] = 1.0
        m = {"befT": befT, "aftT": aftT, "txq": txq_t, "txk": txk_t,
             "txv": txv_t, "bqlw": bqlw, "amask": amask.astype(BF16),
             "hsel": hsel.reshape(8, KT * 128).astype(BF16)}
        m.update(wmats)
        in_maps.append(m)
    return in_maps, scal


def rows_from_outputs(res, scal):
    xt = np.asarray(res["xt"]).reshape(128, NBP, TQ, W2)
    X = np.ascontiguousarray(xt.transpose(2, 0, 1, 3)) \
        .reshape(TQ * 128, NKEY)[0:NQ].astype(np.float32)
    X *= scal["osx"]
    Xr = X.reshape(AL, LV, B, LV)
    t2v = Xr.max(axis=3).sum(axis=1) / LV
    v2t = Xr.max(axis=1).sum(axis=2) / LV
    return t2v, v2t


def finish(results, scal, logit_scale):
    """results: list of 8 per-core {out2, outrm} dicts -> scalar loss."""
    t2v = np.zeros((B, B), np.float64)
    v2t = np.zeros((B, B), np.float64)
    for c in range(NCORES):
        tr, vr = rows_from_outputs(results[c], scal)
        t2v[c * AL:(c + 1) * AL, :] = tr
        v2t[c * AL:(c + 1) * AL, :] = vr
    S = 0.5 * (t2v + v2t) * np.exp(np.float64(np.asarray(logit_scale)))

    def ce(m):
        lse = np.log(np.sum(np.exp(m - m.max(axis=1, keepdims=True)), axis=1)) \
            + m.max(axis=1)
        return -np.mean(np.diag(m) - lse)

    return np.float32(0.5 * (ce(S) + ce(S.T)))


def kernel(**inputs):
    from concourse.bass_utils import run_bass_kernel_spmd

    in_maps, scal = make_in_maps(**inputs)
    nc = get_program(scal)
    res = run_bass_kernel_spmd(nc, in_maps, core_ids=list(range(NCORES)))
    return finish(res.results, scal, inputs["logit_scale"])

